# revision 10
# baseline (speedup 1.0000x reference)
"""SupCon loss kernel for Trainium2 (8 NeuronCores, SPMD row-sharded).

Math (matches the reference):
  S = (E @ E^T) / T,  T = 0.1
  pos_term_i = mean_{j != i, lab_j = lab_i} S_ij
  lse_i      = logsumexp_{j != i} S_ij
  loss       = -sum_i (pos_term_i - lse_i) / N * T

v2 layout (per core c, rows c*1024 .. c*1024+1023):
  - Device computes ONLY the lse path; the positive term is exact host
    math on the same bf16-rounded embeddings (tiny [N,16] matmul).
  - Each core gets a column-ROTATED bf16 E^T (own rows first), so every
    diagonal falls in column-group 0 at offset t*128 for m-tile t and
    the program is identical across cores (pure SPMD).
  - Loop q(seg)-outer / g / t-inner: compute starts after ~1MB of DMA
    and segments 1-3 stream in behind ~27us of matmul per segment.
  - PSUM: [128, 1024] two-bank groups, 4-deep pool (all 8 banks).
    Per group: 8 matmuls (4 k-chunks x 2 halves), one DVE max-reduce,
    one ACT exp (bias=-groupmax, accum_out=group sumexp, bf16 dump).
    Halves the per-chunk DVE/ACT instruction-overhead of v1.
  - Diagonal mask: in-place [128,128] add of diagc (-1e30 on diag) on
    the group-0 psum slice.
  - Per m-tile combine: rowmax over 8 group maxes, rescale group sums.
  - Output per core: [128, MT, 2] f32 = (-rowmax_i, sumexp_i).
Host: lse = -out0 + log(out1); loss = -(pos_total - sum lse) / N * T.
"""

import os
import sys

import numpy as np

for _p in (
    "/root/.axon_site",
    "/root/.axon_site/_ro/trn_rl_repo",
    "/root/.axon_site/_ro/pypackages",
    "/opt/trn_rl_repo",
):
    if os.path.isdir(_p) and _p not in sys.path:
        sys.path.append(_p)

import ml_dtypes

N, D, NCLS, NCORES = 8192, 512, 16, 8
ROWS = N // NCORES        # 1024 rows per core
MT = ROWS // 128          # 8 m-tiles per core
TEMP = 0.1
SCALE = 1.0 / TEMP        # 10.0
KC = D // 128             # 4 k-chunks
SEG = 2048                # DMA segment width for segs 1-3
NSEG = N // SEG           # 4
GW = 1024                 # psum group width (2 banks)
NG = N // GW              # 8 groups per m-tile row
BIG_NEG = -1.0e30

_PROG: dict = {}


def _build_program():
    if "nc" in _PROG:
        return _PROG["nc"]

    import concourse.tile as tile
    from concourse import bacc, mybir

    dt = mybir.dt
    Alu = mybir.AluOpType
    Act = mybir.ActivationFunctionType
    f32, bf16 = dt.float32, dt.bfloat16

    nc = bacc.Bacc("TRN2", target_bir_lowering=False, debug=False)

    etd_d = nc.dram_tensor("etd", [KC, 128, N], bf16, kind="ExternalInput").ap()
    diagc_d = nc.dram_tensor("diagc", [128, 128], f32, kind="ExternalInput").ap()
    out_d = nc.dram_tensor("out_vals", [128, MT, 2], f32, kind="ExternalOutput").ap()

    with tile.TileContext(nc) as tc:
        with (
            tc.tile_pool(name="consts", bufs=1) as consts,
            tc.tile_pool(name="ets", bufs=1) as ets,
            tc.tile_pool(name="dump", bufs=2) as dump,
            tc.tile_pool(name="small", bufs=2) as small,
            tc.tile_pool(name="acc", bufs=1) as accp,
            tc.tile_pool(name="psum", bufs=4, space="PSUM") as psum,
        ):
            # ALL input DMAs go on nc.sync: sync-issued transfers ride the
            # fast software-dynamic descriptor path (~146GB/s aggregate);
            # gpsimd-issued ones fall onto hardware-dynamic queues that
            # trickle at ~30GB/s and pace the whole kernel.
            # seg 0 as 16 [128,512] tiles, j-major k-inner so the first
            # matmul (needs et0[0][0] as both lhsT and rhs) waits on one
            # 128KB DMA only.
            et0 = [[None] * 4 for _ in range(KC)]
            diagc = consts.tile([128, 128], f32)
            for j in range(4):
                for k in range(KC):
                    e0t = ets.tile([128, 512], bf16, name=f"et0_{k}_{j}")
                    nc.sync.dma_start(e0t[:], etd_d[k, :, j * 512 : (j + 1) * 512])
                    et0[k][j] = e0t
                    if j == 0 and k == 0:
                        # first matmul only needs et0[0][0]; diagc is not
                        # needed until ~3us later -- issue it second
                        nc.sync.dma_start(diagc[:], diagc_d[:])
            et = [[None] * NSEG for _ in range(KC)]
            for s in range(1, NSEG):
                for k in range(KC):
                    ektile = ets.tile([128, SEG], bf16, name=f"et_{k}_{s}")
                    nc.sync.dma_start(ektile[:], etd_d[k, :, s * SEG : (s + 1) * SEG])
                    et[k][s] = ektile

            gmaxs = accp.tile([128, MT, NG], f32)   # negated group maxes
            gsums = accp.tile([128, MT, NG], f32)   # per-group sumexp
            vals = accp.tile([128, MT, 2], f32)

            def rhs_half(q, g, h, k):
                j512 = g * 2 + h
                if q == 0:
                    return et0[k][j512][:]
                return et[k][q][:, j512 * 512 : (j512 + 1) * 512]

            for q in range(NSEG):
                for g in range(2):
                    gi = q * 2 + g
                    for t in range(MT):
                        ps = psum.tile([128, GW], f32, tag="ps")
                        lj, lo = t // 4, (t % 4) * 128
                        for k in range(KC):
                            lhsT = et0[k][lj][:, lo : lo + 128]
                            for h in range(2):
                                nc.tensor.matmul(
                                    ps[:, h * 512 : (h + 1) * 512],
                                    lhsT,
                                    rhs_half(q, g, h, k),
                                    start=(k == 0),
                                    stop=(k == KC - 1),
                                )
                        if gi == 0:
                            # rotated diag of m-tile t sits at cols
                            # t*128..t*128+127 of group 0 -> mask in place
                            dsl = ps[:, t * 128 : (t + 1) * 128]
                            nc.vector.scalar_tensor_tensor(
                                out=dsl,
                                in0=dsl,
                                scalar=1.0,
                                in1=diagc[:],
                                op0=Alu.mult,
                                op1=Alu.add,
                            )
                        nm = gmaxs[:, t, gi : gi + 1]
                        nc.vector.tensor_reduce(
                            nm, ps[:], axis=mybir.AxisListType.X, op=Alu.max,
                            negate=True,
                        )
                        ed = dump.tile([128, GW], bf16, tag="ed")
                        nc.scalar.activation(
                            ed[:], ps[:], Act.Exp, bias=nm, scale=1.0,
                            accum_out=gsums[:, t, gi : gi + 1],
                        )

            for t in range(MT):
                negb = small.tile([128, 1], f32, tag="negb")    # -rowmax
                nc.vector.tensor_reduce(
                    negb[:], gmaxs[:, t, :], axis=mybir.AxisListType.X,
                    op=Alu.min,
                )
                e8 = small.tile([128, NG], f32, tag="e8")
                nc.scalar.activation(
                    e8[:], gmaxs[:, t, :], Act.Exp, bias=negb[:], scale=-1.0
                )
                t8 = small.tile([128, NG], f32, tag="t8")
                nc.vector.scalar_tensor_tensor(
                    out=t8[:],
                    in0=gsums[:, t, :],
                    scalar=1.0,
                    in1=e8[:],
                    op0=Alu.mult,
                    op1=Alu.mult,
                    accum_out=vals[:, t, 1:2],
                )
                nc.vector.tensor_copy(vals[:, t, 0:1], negb[:])

            nc.sync.dma_start(out_d[:], vals[:])

    nc.compile()
    _PROG["nc"] = nc
    return nc


def _prep_inputs(embeddings: np.ndarray, labels: np.ndarray):
    E = np.asarray(embeddings, dtype=np.float32)
    lab = np.asarray(labels).astype(np.int64)
    assert E.shape == (N, D) and lab.shape == (N,)

    # pre-scale by sqrt(1/T) so PSUM dots are already in S-units
    Ebf = (E * np.float32(np.sqrt(SCALE))).astype(ml_dtypes.bfloat16)
    Ef = Ebf.astype(np.float64)

    # exact host positive term from the same bf16-rounded E the device
    # sees: pos_i = (e_i . g_{lab_i} - ||e_i||^2) / (cnt_i - 1), S-units
    G = np.zeros((D, NCLS), np.float64)
    for l in range(NCLS):
        G[:, l] = Ef[lab == l].sum(axis=0)
    C = Ef @ G                                     # [N, NCLS]
    cnt = np.bincount(lab, minlength=NCLS).astype(np.float64)
    selfdot = (Ef * Ef).sum(axis=1)
    pos = (C[np.arange(N), lab] - selfdot) / (cnt[lab] - 1.0)
    pos_total = float(pos.sum())

    ET = np.ascontiguousarray(Ebf.T)               # [D, N] bf16

    diagc = np.zeros((128, 128), np.float32)
    diagc[np.arange(128), np.arange(128)] = BIG_NEG

    in_maps = []
    for c in range(NCORES):
        rot = np.roll(ET, -c * ROWS, axis=1)       # own columns first
        etd = np.ascontiguousarray(rot.reshape(KC, 128, N))
        in_maps.append({"etd": etd, "diagc": diagc})
    return in_maps, pos_total


def run(embeddings, labels, trace=False, tmpdir=None):
    """Build+run on 8 cores; returns (loss_scalar, BassKernelResults)."""
    from concourse.bass_utils import run_bass_kernel_spmd

    nc = _build_program()
    in_maps, pos_total = _prep_inputs(embeddings, labels)
    res = run_bass_kernel_spmd(
        nc, in_maps, list(range(NCORES)), trace=trace, tmpdir=tmpdir
    )
    lse_total = 0.0
    for r in res.results:
        ov = r["out_vals"].astype(np.float64)
        # lse = rowmax + log(sumexp) = -out0 + log(out1)
        lse_total += float((-ov[:, :, 0] + np.log(ov[:, :, 1])).sum())
    loss = -(pos_total - lse_total) / N * TEMP
    return np.float32(loss), res


def kernel(**inputs) -> np.ndarray:
    loss, _ = run(inputs["embeddings"], inputs["labels"])
    return loss


# revision 11
# speedup vs baseline: 1.0166x; 1.0166x over previous
"""SupCon loss kernel for Trainium2 (8 NeuronCores, SPMD row-sharded).

Math (matches the reference):
  S = (E @ E^T) / T,  T = 0.1
  pos_term_i = mean_{j != i, lab_j = lab_i} S_ij
  lse_i      = logsumexp_{j != i} S_ij
  loss       = -sum_i (pos_term_i - lse_i) / N * T

v2 layout (per core c, rows c*1024 .. c*1024+1023):
  - Device computes ONLY the lse path; the positive term is exact host
    math on the same bf16-rounded embeddings (tiny [N,16] matmul).
  - Each core gets a column-ROTATED bf16 E^T (own rows first), so every
    diagonal falls in column-group 0 at offset t*128 for m-tile t and
    the program is identical across cores (pure SPMD).
  - Loop q(seg)-outer / g / t-inner: compute starts after ~1MB of DMA
    and segments 1-3 stream in behind ~27us of matmul per segment.
  - PSUM: [128, 1024] two-bank groups, 4-deep pool (all 8 banks).
    Per group: 8 matmuls (4 k-chunks x 2 halves), one DVE max-reduce,
    one ACT exp (bias=-groupmax, accum_out=group sumexp, bf16 dump).
    Halves the per-chunk DVE/ACT instruction-overhead of v1.
  - Diagonal mask: in-place [128,128] add of diagc (-1e30 on diag) on
    the group-0 psum slice.
  - Per m-tile combine: rowmax over 8 group maxes, rescale group sums.
  - Output per core: [128, MT, 2] f32 = (-rowmax_i, sumexp_i).
Host: lse = -out0 + log(out1); loss = -(pos_total - sum lse) / N * T.
"""

import os
import sys

import numpy as np

for _p in (
    "/root/.axon_site",
    "/root/.axon_site/_ro/trn_rl_repo",
    "/root/.axon_site/_ro/pypackages",
    "/opt/trn_rl_repo",
):
    if os.path.isdir(_p) and _p not in sys.path:
        sys.path.append(_p)

import ml_dtypes

N, D, NCLS, NCORES = 8192, 512, 16, 8
ROWS = N // NCORES        # 1024 rows per core
MT = ROWS // 128          # 8 m-tiles per core
TEMP = 0.1
SCALE = 1.0 / TEMP        # 10.0
KC = D // 128             # 4 k-chunks
SEG = 2048                # DMA segment width for segs 1-3
NSEG = N // SEG           # 4
GW = 1024                 # psum group width (2 banks)
NG = N // GW              # 8 groups per m-tile row
BIG_NEG = -1.0e30

_PROG: dict = {}


def _build_program():
    if "nc" in _PROG:
        return _PROG["nc"]

    import concourse.tile as tile
    from concourse import bacc, mybir

    dt = mybir.dt
    Alu = mybir.AluOpType
    Act = mybir.ActivationFunctionType
    f32, bf16 = dt.float32, dt.bfloat16

    nc = bacc.Bacc("TRN2", target_bir_lowering=False, debug=False)

    etd_d = nc.dram_tensor("etd", [KC, 128, N], bf16, kind="ExternalInput").ap()
    diagc_d = nc.dram_tensor("diagc", [128, 128], f32, kind="ExternalInput").ap()
    out_d = nc.dram_tensor("out_vals", [128, MT, 2], f32, kind="ExternalOutput").ap()

    with tile.TileContext(nc) as tc:
        with (
            tc.tile_pool(name="consts", bufs=1) as consts,
            tc.tile_pool(name="ets", bufs=1) as ets,
            tc.tile_pool(name="dump", bufs=2) as dump,
            tc.tile_pool(name="small", bufs=2) as small,
            tc.tile_pool(name="acc", bufs=1) as accp,
            tc.tile_pool(name="psum", bufs=4, space="PSUM") as psum,
        ):
            # ALL input DMAs go on nc.sync: sync-issued transfers ride the
            # fast software-dynamic descriptor path (~146GB/s aggregate);
            # gpsimd-issued ones fall onto hardware-dynamic queues that
            # trickle at ~30GB/s and pace the whole kernel.
            # seg 0 as 16 [128,512] tiles, j-major k-inner so the first
            # matmul (needs et0[0][0] as both lhsT and rhs) waits on one
            # 128KB DMA only.
            et0 = [[None] * 4 for _ in range(KC)]
            diagc = consts.tile([128, 128], f32)
            nc.sync.dma_start(diagc[:], diagc_d[:])
            for j in range(4):
                for k in range(KC):
                    e0t = ets.tile([128, 512], bf16, name=f"et0_{k}_{j}")
                    nc.sync.dma_start(e0t[:], etd_d[k, :, j * 512 : (j + 1) * 512])
                    et0[k][j] = e0t
            et = [[None] * NSEG for _ in range(KC)]
            for s in range(1, NSEG):
                for k in range(KC):
                    ektile = ets.tile([128, SEG], bf16, name=f"et_{k}_{s}")
                    nc.sync.dma_start(ektile[:], etd_d[k, :, s * SEG : (s + 1) * SEG])
                    et[k][s] = ektile

            gmaxs = accp.tile([128, MT, NG], f32)   # negated group maxes
            gsums = accp.tile([128, MT, NG], f32)   # per-group sumexp
            vals = accp.tile([128, MT, 2], f32)

            def rhs_half(q, g, h, k):
                j512 = g * 2 + h
                if q == 0:
                    return et0[k][j512][:]
                return et[k][q][:, j512 * 512 : (j512 + 1) * 512]

            for q in range(NSEG):
                for g in range(2):
                    gi = q * 2 + g
                    for t in range(MT):
                        ps = psum.tile([128, GW], f32, tag="ps")
                        lj, lo = t // 4, (t % 4) * 128
                        for k in range(KC):
                            lhsT = et0[k][lj][:, lo : lo + 128]
                            for h in range(2):
                                nc.tensor.matmul(
                                    ps[:, h * 512 : (h + 1) * 512],
                                    lhsT,
                                    rhs_half(q, g, h, k),
                                    start=(k == 0),
                                    stop=(k == KC - 1),
                                )
                        if gi == 0:
                            # rotated diag of m-tile t sits at cols
                            # t*128..t*128+127 of group 0 -> mask in place
                            dsl = ps[:, t * 128 : (t + 1) * 128]
                            nc.vector.scalar_tensor_tensor(
                                out=dsl,
                                in0=dsl,
                                scalar=1.0,
                                in1=diagc[:],
                                op0=Alu.mult,
                                op1=Alu.add,
                            )
                        nm = gmaxs[:, t, gi : gi + 1]
                        nc.vector.tensor_reduce(
                            nm, ps[:], axis=mybir.AxisListType.X, op=Alu.max,
                            negate=True,
                        )
                        ed = dump.tile([128, GW], bf16, tag="ed")
                        nc.scalar.activation(
                            ed[:], ps[:], Act.Exp, bias=nm, scale=1.0,
                            accum_out=gsums[:, t, gi : gi + 1],
                        )

            for t in range(MT):
                negb = small.tile([128, 1], f32, tag="negb")    # -rowmax
                nc.vector.tensor_reduce(
                    negb[:], gmaxs[:, t, :], axis=mybir.AxisListType.X,
                    op=Alu.min,
                )
                e8 = small.tile([128, NG], f32, tag="e8")
                nc.scalar.activation(
                    e8[:], gmaxs[:, t, :], Act.Exp, bias=negb[:], scale=-1.0
                )
                t8 = small.tile([128, NG], f32, tag="t8")
                nc.vector.scalar_tensor_tensor(
                    out=t8[:],
                    in0=gsums[:, t, :],
                    scalar=1.0,
                    in1=e8[:],
                    op0=Alu.mult,
                    op1=Alu.mult,
                    accum_out=vals[:, t, 1:2],
                )
                nc.vector.tensor_copy(vals[:, t, 0:1], negb[:])

            nc.sync.dma_start(out_d[:], vals[:])

    nc.compile()
    _PROG["nc"] = nc
    return nc


def _prep_inputs(embeddings: np.ndarray, labels: np.ndarray):
    E = np.asarray(embeddings, dtype=np.float32)
    lab = np.asarray(labels).astype(np.int64)
    assert E.shape == (N, D) and lab.shape == (N,)

    # pre-scale by sqrt(1/T) so PSUM dots are already in S-units
    Ebf = (E * np.float32(np.sqrt(SCALE))).astype(ml_dtypes.bfloat16)
    Ef = Ebf.astype(np.float64)

    # exact host positive term from the same bf16-rounded E the device
    # sees: pos_i = (e_i . g_{lab_i} - ||e_i||^2) / (cnt_i - 1), S-units
    G = np.zeros((D, NCLS), np.float64)
    for l in range(NCLS):
        G[:, l] = Ef[lab == l].sum(axis=0)
    C = Ef @ G                                     # [N, NCLS]
    cnt = np.bincount(lab, minlength=NCLS).astype(np.float64)
    selfdot = (Ef * Ef).sum(axis=1)
    pos = (C[np.arange(N), lab] - selfdot) / (cnt[lab] - 1.0)
    pos_total = float(pos.sum())

    ET = np.ascontiguousarray(Ebf.T)               # [D, N] bf16

    diagc = np.zeros((128, 128), np.float32)
    diagc[np.arange(128), np.arange(128)] = BIG_NEG

    in_maps = []
    for c in range(NCORES):
        rot = np.roll(ET, -c * ROWS, axis=1)       # own columns first
        etd = np.ascontiguousarray(rot.reshape(KC, 128, N))
        in_maps.append({"etd": etd, "diagc": diagc})
    return in_maps, pos_total


def run(embeddings, labels, trace=False, tmpdir=None):
    """Build+run on 8 cores; returns (loss_scalar, BassKernelResults)."""
    from concourse.bass_utils import run_bass_kernel_spmd

    nc = _build_program()
    in_maps, pos_total = _prep_inputs(embeddings, labels)
    res = run_bass_kernel_spmd(
        nc, in_maps, list(range(NCORES)), trace=trace, tmpdir=tmpdir
    )
    lse_total = 0.0
    for r in res.results:
        ov = r["out_vals"].astype(np.float64)
        # lse = rowmax + log(sumexp) = -out0 + log(out1)
        lse_total += float((-ov[:, :, 0] + np.log(ov[:, :, 1])).sum())
    loss = -(pos_total - lse_total) / N * TEMP
    return np.float32(loss), res


def kernel(**inputs) -> np.ndarray:
    loss, _ = run(inputs["embeddings"], inputs["labels"])
    return loss
